# revision 1
# baseline (speedup 1.0000x reference)
"""Trainium2 Bass kernel for a 64-component mixed spherical (vMF) gaussian
distribution evaluated at 1M unit directions.

    out[s] = sum_n lambda_n * C(kappa_n) * exp(kappa_n * (dot(wi_s, mu_n) - 1))

Strategy (per core, data-parallel over S across 8 cores):
  * components n=0..63 live on SBUF/PSUM partitions; samples on the free dim
  * two half-streams of samples are packed block-diagonally so all 128
    partitions are used:  partitions 0:64  -> samples [0, S/2)
                          partitions 64:128-> samples [S/2, S)
  * TensorE:  dotk[p, s] = kappa_n * dot(wi_s, mu_n).  K=18+18 block-diag
    contraction (bf16 hi/lo split of A = kappa*mu and of wi).  Four 512-col
    sample tiles run CONCURRENTLY on the four 32-row PE strips
    (tile_position=(32q, 0)) — the dot costs almost nothing on PE.
  * ScalarE:  pdf[p, s] = Exp(dotk + bias_p)  with per-partition
    bias_n = log(lambda_n * C(kappa_n)) - kappa_n  (free affine of ACT).
    This is the bottleneck engine: 8.4M exps/core at 1 elem/lane/cycle.
  * TensorE:  cross-partition reduction via a sliding one-hot window,
    accumulated into a [128, 512] PSUM bank per 64-tile group; results
    rotate over the four 32-col PE strips (tile_position=(0, 32b)).
    Reduce matmuls are emitted 2 blocks late so PE's strict-FIFO queue
    never blocks the next dot behind an Exp-gated reduce.
  * DVE copies the accumulated bank to SBUF; DMA to HBM; host de-leaves.

Exp blocks use a mixed (3x10, 2)-tile plan per 32-tile chunk: 44 ACT
instructions per pass instead of 64 (ACT per-instruction overhead is the
only cost above the exp-stream floor; 3-bank dot tiles x2 bufs + 2
reduction banks exactly fill the 8 PSUM banks).

Measured ~57-60 us/core steady-state body in clean windows (repeat-slope
method on HW; loaded windows read up to ~75) vs the 54.6 us exp-stream
roofline; cost-model sim 74.0 us e2e.  Optimization history: 116 us
(first correct) -> 80 (row-tiled dot) -> 74 (lagged reductions) -> ~72
(padded DMA, warm-up) -> ~58 (mixed big exp blocks).
"""

import math
import numpy as np
import ml_dtypes

N_COMP = 64
N_DIRS = 1048576
N_CORES = 8
S_LOCAL = N_DIRS // N_CORES      # 131072 samples per core
S_HALF = S_LOCAL // 2            # 65536 per half-stream
TILE_N = 512                     # matmul moving free dim (one PSUM bank fp32)
BLOCK = 1024                     # columns per ACT instruction / psum tile
CHUNK = 16384                    # wi columns per input DMA
GROUP = 64 * TILE_N              # 32768 columns whose reductions share a bank
N_GROUPS = S_HALF // GROUP       # 2

BF16 = ml_dtypes.bfloat16

_CACHED_NC = None


def _build_bass(repeat=1, mode="full", red_lag=2):
    import concourse.bacc as bacc
    import concourse.tile as tile
    from concourse import mybir

    if mode.startswith("act") and mode != "act_only":
        return _build_act_bench(int(mode[3:]), repeat)
    do_dot = mode in ("full", "dot_act", "dot_only")
    do_act = mode in ("full", "dot_act", "act_only")
    do_red = mode == "full"

    nc = bacc.Bacc("TRN2", target_bir_lowering=False, debug=False,
                   num_devices=N_CORES)

    # wi4: 512-column sample-tile t lives on partition strip 32*(t%4)+[0,18)
    # at columns [(t//4)*512, (t//4+1)*512) — four tiles are processed
    # concurrently by row-tiled matmuls on the four 32-row PE strips.
    # (Rows 32q+18..32q+31 are zero padding; one big DMA per chunk measured
    # faster than four packed strip-DMAs.)
    wi4 = nc.dram_tensor("wi4", [128, S_HALF // 4], mybir.dt.bfloat16,
                         kind="ExternalInput")
    lhs_dot = nc.dram_tensor("lhs_dot", [128, 128], mybir.dt.bfloat16,
                             kind="ExternalInput")
    # Reduction weights, sliding 32-wide window: only columns 32/33 are
    # nonzero (ones over partitions [0,64) / [64,128)).  For reduce-tile j
    # (i = j%16, b = j//16) the slice big_red[:, 32-2i : 64-2i] is a
    # [128, 32] matrix whose column 2i selects the first-half sum and 2i+1
    # the second-half sum; the output goes to the 32-aligned PSUM strip
    # [32b, 32b+32).  Narrow windows keep LDWEIGHTS cheap (cost ~ columns).
    big_red = nc.dram_tensor("big_red", [128, 64], mybir.dt.bfloat16,
                             kind="ExternalInput")
    bias = nc.dram_tensor("bias", [128, 1], mybir.dt.float32,
                          kind="ExternalInput")
    # raw[g, p, i]: group g, PSUM partition p = 2*j + h (reduce-tile j,
    # half-stream h), column i.  Host de-interleaves.
    out = nc.dram_tensor("out", [N_GROUPS, 128, TILE_N], mybir.dt.float32,
                         kind="ExternalOutput")

    fp32 = mybir.dt.float32
    bf16 = mybir.dt.bfloat16

    with tile.TileContext(nc) as tc:
        with (
            tc.tile_pool(name="consts", bufs=1) as consts,
            tc.tile_pool(name="wi", bufs=3) as wi_pool,
            tc.tile_pool(name="pdf", bufs=6) as pdf_pool,
            tc.tile_pool(name="outsb", bufs=2) as out_pool,
            tc.tile_pool(name="dot_ps", bufs=2, space="PSUM") as dot_pool,
            tc.tile_pool(name="red_ps", bufs=2, space="PSUM") as red_pool,
        ):
            lhs_dot_sb = consts.tile([128, 128], bf16)
            nc.sync.dma_start(out=lhs_dot_sb[:], in_=lhs_dot[:])
            big_red_sb = consts.tile([128, 64], bf16)
            nc.sync.dma_start(out=big_red_sb[:], in_=big_red[:])
            bias_sb = consts.tile([128, 1], fp32)
            nc.sync.dma_start(out=bias_sb[:], in_=bias[:])

            # Dependency-free dummy exp so the ~2.7us ACT table load runs
            # at t=0, concurrent with the first DMAs/matmuls.
            warm = consts.tile([1, 8], fp32)
            nc.vector.memset(warm[:], 0.0)
            nc.scalar.activation(warm[:], warm[:],
                                 mybir.ActivationFunctionType.Exp)

            stat_t = None
            if not do_dot:
                wi0 = consts.tile([128, TILE_N], bf16)
                nc.sync.dma_start(out=wi0[:], in_=wi4[:, 0:TILE_N])
                stat_t = dot_pool.tile([128, BLOCK], fp32)
                for mi in range(BLOCK // TILE_N):
                    nc.tensor.matmul(
                        stat_t[:, mi * TILE_N:(mi + 1) * TILE_N],
                        lhs_dot_sb[0:18, :], wi0[0:18, 0:TILE_N],
                        start=True, stop=True, tile_position=(0, 0))

            # Reduction matmuls are emitted RED_LAG blocks behind the
            # dot/exp pipeline: PE's queue is strict FIFO, so a reduce
            # waiting on Exp(i) must not sit in front of dot(i+1).
            RED_LAG = red_lag
            pending = []        # (pdf_t, gr, [tile indices within group])
            red_map = {}        # gr -> red accumulation psum tile
            last_pdf = None
            last_dot = stat_t

            def emit_reds(pdf_t, gr, tiles):
                g = gr % N_GROUPS
                if gr not in red_map:
                    red_map[gr] = red_pool.tile([128, TILE_N], fp32,
                                                name="red_t", tag="red_t")
                red_t = red_map[gr]
                for mi, t in enumerate(tiles):
                    b, i = t % 4, t // 4
                    nc.tensor.matmul(
                        red_t[32 * b:32 * b + 32, :],
                        big_red_sb[:, 32 - 2 * i:64 - 2 * i],
                        pdf_t[:, mi * TILE_N:(mi + 1) * TILE_N],
                        start=(i == 0), stop=(i == 15),
                        skip_group_check=True,
                        tile_position=(0, 32 * b),
                    )
                if tiles[-1] == GROUP // TILE_N - 1:
                    out_sb = out_pool.tile([128, TILE_N], fp32)
                    nc.vector.tensor_copy(out_sb[:], red_t[:])
                    nc.sync.dma_start(out=out[g], in_=out_sb[:])
                    del red_map[gr]

            for gr in range(N_GROUPS * repeat):
                g = gr % N_GROUPS
                for ci in range(GROUP // CHUNK):
                    first = gr == 0 and ci == 0
                    if do_dot and not first:
                        wi_t = wi_pool.tile([128, CHUNK // 4], bf16)
                        col0 = (g * GROUP + ci * CHUNK) // 4
                        nc.sync.dma_start(out=wi_t[:],
                                          in_=wi4[:, col0:col0 + CHUNK // 4])
                    elif do_dot:
                        # Kernel warm-up: fetch the very first chunk in four
                        # small pieces so the first matmuls/exps start ~4us
                        # earlier instead of waiting for one 512 KiB DMA.
                        subs = []
                        for si in range(8):
                            wi_s = wi_pool.tile([128, TILE_N], bf16,
                                                name=f"wi_first{si}",
                                                tag=f"wi_first{si}")
                            nc.sync.dma_start(
                                out=wi_s[:],
                                in_=wi4[:, si * TILE_N:(si + 1) * TILE_N])
                            subs.append(wi_s)
                    # Mixed block plan per 32-tile chunk: ten 3-tile
                    # blocks + one 2-tile block -> 11 Exp instructions per
                    # chunk instead of 16 (ACT per-instruction overhead is
                    # the only cost above the exp-stream floor).
                    tc0 = 0
                    for blen in (3, 3, 3, 3, 3, 3, 3, 3, 3, 3, 2):
                        tiles_c = list(range(tc0, tc0 + blen))
                        tc0 += blen
                        if do_dot:
                            dot_t = dot_pool.tile([128, blen * TILE_N], fp32,
                                                  name="dot_t", tag="dot_t")
                            for mi, t_c in enumerate(tiles_c):
                                q = t_c % 4
                                if first:
                                    wi_cur, u0 = subs[t_c // 4], 0
                                else:
                                    wi_cur, u0 = wi_t, (t_c // 4) * TILE_N
                                nc.tensor.matmul(
                                    dot_t[:, mi * TILE_N:(mi + 1) * TILE_N],
                                    lhs_dot_sb[32 * q:32 * q + 18, :],
                                    wi_cur[32 * q:32 * q + 18, u0:u0 + TILE_N],
                                    start=True, stop=True,
                                    tile_position=(32 * q, 0),
                                )
                            last_dot = dot_t
                        else:
                            dot_t = stat_t
                        if do_act:
                            pdf_t = pdf_pool.tile([128, blen * TILE_N], bf16,
                                                  name="pdf_t", tag="pdf_t")
                            nc.scalar.activation(
                                pdf_t[:], dot_t[:, 0:blen * TILE_N],
                                mybir.ActivationFunctionType.Exp,
                                bias=bias_sb[:, 0:1], scale=1.0,
                            )
                            last_pdf = pdf_t
                        if do_red:
                            base = ci * (CHUNK // TILE_N)
                            pending.append(
                                (pdf_t, gr, [base + t for t in tiles_c]))
                            if len(pending) > RED_LAG:
                                emit_reds(*pending.pop(0))
            while pending:
                emit_reds(*pending.pop(0))

            if not do_red:
                red_t = red_pool.tile([128, TILE_N], fp32)
                if last_pdf is not None:
                    nc.tensor.matmul(red_t[0:32, :], big_red_sb[:, 32:64],
                                     last_pdf[:, 0:TILE_N],
                                     start=True, stop=True,
                                     tile_position=(0, 0))
                for g in range(N_GROUPS):
                    out_sb = out_pool.tile([128, TILE_N], fp32)
                    csrc = red_t if last_pdf is not None else last_dot
                    nc.vector.tensor_copy(out_sb[:], csrc[:, 0:TILE_N])
                    nc.sync.dma_start(out=out[g], in_=out_sb[:])

    nc.compile()
    return nc


def _build_act_bench(block, repeat):
    """ACT-only throughput probe: back-to-back Exp over a static [128, block]
    PSUM tile, same per-pass element count as the real kernel."""
    import concourse.bacc as bacc
    import concourse.tile as tile
    from concourse import mybir

    nc = bacc.Bacc("TRN2", target_bir_lowering=False, debug=False,
                   num_devices=N_CORES)
    wi4 = nc.dram_tensor("wi4", [128, S_HALF // 4], mybir.dt.bfloat16,
                         kind="ExternalInput")
    lhs_dot = nc.dram_tensor("lhs_dot", [128, 128], mybir.dt.bfloat16,
                             kind="ExternalInput")
    big_red = nc.dram_tensor("big_red", [128, 64], mybir.dt.bfloat16,
                             kind="ExternalInput")
    bias = nc.dram_tensor("bias", [128, 1], mybir.dt.float32,
                          kind="ExternalInput")
    out = nc.dram_tensor("out", [N_GROUPS, 128, TILE_N], mybir.dt.float32,
                         kind="ExternalOutput")
    fp32, bf16 = mybir.dt.float32, mybir.dt.bfloat16
    n_act = (S_HALF + block - 1) // block   # per pass

    with tile.TileContext(nc) as tc:
        with (
            tc.tile_pool(name="consts", bufs=1) as consts,
            tc.tile_pool(name="pdf", bufs=4) as pdf_pool,
            tc.tile_pool(name="outsb", bufs=2) as out_pool,
            tc.tile_pool(name="stat_ps", bufs=1, space="PSUM") as stat_pool,
        ):
            lhs_dot_sb = consts.tile([128, 128], bf16)
            nc.sync.dma_start(out=lhs_dot_sb[:], in_=lhs_dot[:])
            bias_sb = consts.tile([128, 1], fp32)
            nc.sync.dma_start(out=bias_sb[:], in_=bias[:])
            wi0 = consts.tile([128, TILE_N], bf16)
            for q in range(4):
                nc.sync.dma_start(out=wi0[32 * q:32 * q + 18, :],
                                  in_=wi4[18 * q:18 * q + 18, 0:TILE_N])
            stat_t = stat_pool.tile([128, block], fp32)
            for mi in range(block // TILE_N):
                nc.tensor.matmul(
                    stat_t[:, mi * TILE_N:(mi + 1) * TILE_N],
                    lhs_dot_sb[0:18, :], wi0[0:18, :],
                    start=True, stop=True, tile_position=(0, 0))
            warm = consts.tile([1, 8], fp32)
            nc.vector.memset(warm[:], 0.0)
            nc.scalar.activation(warm[:], warm[:],
                                 mybir.ActivationFunctionType.Exp)

            for gr in range(repeat):
                last_pdf = None
                for _ in range(n_act):
                    pdf_t = pdf_pool.tile([128, block], bf16)
                    nc.scalar.activation(
                        pdf_t[:], stat_t[:],
                        mybir.ActivationFunctionType.Exp,
                        bias=bias_sb[:, 0:1], scale=1.0)
                    last_pdf = pdf_t
                out_sb = out_pool.tile([128, TILE_N], fp32)
                nc.vector.tensor_copy(out_sb[:], last_pdf[:, 0:TILE_N])
                for g in range(N_GROUPS):
                    nc.sync.dma_start(out=out[g], in_=out_sb[:])

    nc.compile()
    return nc


def _get_nc(repeat=1):
    global _CACHED_NC
    if repeat != 1:
        return _build_bass(repeat=repeat)
    if _CACHED_NC is None:
        _CACHED_NC = _build_bass()
    return _CACHED_NC


def _host_prep(lambdas, kappas, thetas, phis, wi):
    """Build per-core input maps (tiny O(64) parameter math + bf16 hi/lo
    split and layout of wi)."""
    lambdas = np.asarray(lambdas, np.float32)
    kappas = np.asarray(kappas, np.float32)
    thetas = np.asarray(thetas, np.float32)
    phis = np.asarray(phis, np.float32)
    wi = np.ascontiguousarray(np.asarray(wi, np.float32))

    # spherical -> cartesian mean directions, scaled by kappa
    st = np.sin(thetas)
    mu = np.stack([st * np.cos(phis), st * np.sin(phis), np.cos(thetas)],
                  axis=-1).astype(np.float32)          # [64, 3]
    A = (mu * kappas[:, None]).astype(np.float32)      # [64, 3]
    A1 = A.astype(BF16)
    A2 = (A - A1.astype(np.float32)).astype(BF16)

    # vMF normalization (mirrors reference._vmf_norm, fp32)
    k = np.maximum(kappas, np.float32(1e-8))
    with np.errstate(divide="ignore", over="ignore", invalid="ignore"):
        norm_k = np.where(
            kappas < np.float32(1e-5),
            np.float32(1.0 / (4.0 * math.pi)),
            k * np.float32(1.0 / (2.0 * math.pi))
            / (np.float32(1.0) - np.exp(-2.0 * k).astype(np.float32)),
        ).astype(np.float32)
    bias64 = (np.log(lambdas * norm_k) - kappas).astype(np.float32)   # [64]
    bias128 = np.concatenate([bias64, bias64]).reshape(128, 1)

    # lhs for the dot matmul: block-diagonal bf16 hi/lo split of A
    # pairing rows: (A1,B1) (A1,B2) (A2,B1) over the 3 dims each;
    # replicated on the four 32-row PE strips for row-tiled matmuls
    A9 = np.concatenate([A1.T, A1.T, A2.T], axis=0)    # [9, 64] bf16
    lhs18 = np.zeros((18, 128), BF16)
    lhs18[0:9, 0:64] = A9
    lhs18[9:18, 64:128] = A9
    lhs_dot = np.zeros((128, 128), BF16)
    for q in range(4):
        lhs_dot[32 * q:32 * q + 18, :] = lhs18

    # lhs for the reduction matmul: sliding-window one-hot block
    big_red = np.zeros((128, 64), BF16)
    big_red[0:64, 32] = BF16(1.0)
    big_red[64:128, 33] = BF16(1.0)

    # wi bf16 hi/lo split, paired to match lhs rows
    B1 = wi.astype(BF16)                               # [S, 3]
    B2 = (wi - B1.astype(np.float32)).astype(BF16)
    B9 = np.concatenate([B1.T, B2.T, B1.T], axis=0)    # [9, S] bf16

    in_maps = []
    for c in range(N_CORES):
        c0 = c * S_LOCAL
        wi18 = np.empty((18, S_HALF), BF16)
        wi18[0:9] = B9[:, c0:c0 + S_HALF]
        wi18[9:18] = B9[:, c0 + S_HALF:c0 + S_LOCAL]
        # scatter 512-col sample tiles over the four PE row strips
        arr = wi18.reshape(18, S_HALF // TILE_N, TILE_N)
        wi4 = np.zeros((128, S_HALF // 4), BF16)
        for q in range(4):
            wi4[32 * q:32 * q + 18] = arr[:, q::4, :].reshape(18, S_HALF // 4)
        in_maps.append({
            "wi4": wi4,
            "lhs_dot": lhs_dot,
            "big_red": big_red,
            "bias": bias128,
        })
    return in_maps


def _assemble(results):
    out = np.empty(N_DIRS, np.float32)
    for c in range(N_CORES):
        r = np.asarray(results[c]["out"], np.float32)   # [N_GROUPS, 128, 512]
        # PSUM partition p = 32*b + 2*i + h for sample tile t = 4*i + b,
        # half-stream h
        r = r.reshape(N_GROUPS, 4, 16, 2, TILE_N)
        c0 = c * S_LOCAL
        out[c0:c0 + S_HALF] = \
            r[:, :, :, 0, :].transpose(0, 2, 1, 3).reshape(S_HALF)
        out[c0 + S_HALF:c0 + S_LOCAL] = \
            r[:, :, :, 1, :].transpose(0, 2, 1, 3).reshape(S_HALF)
    return out


def kernel(**inputs):
    from concourse.bass_utils import run_bass_kernel_spmd

    in_maps = _host_prep(**inputs)
    nc = _get_nc()
    try:
        res = run_bass_kernel_spmd(nc, in_maps, core_ids=list(range(N_CORES)))
    except Exception:
        # one retry for transient device/terminal hiccups
        res = run_bass_kernel_spmd(nc, in_maps, core_ids=list(range(N_CORES)))
    return _assemble(res.results)


def kernel_traced(**inputs):
    """Like kernel() but with NTFF tracing; returns (out, BassKernelResults)."""
    from concourse.bass_utils import run_bass_kernel_spmd

    in_maps = _host_prep(**inputs)
    nc = _get_nc()
    res = run_bass_kernel_spmd(nc, in_maps, core_ids=list(range(N_CORES)),
                               trace=True)
    return _assemble(res.results), res



# revision 2
# speedup vs baseline: 1.0481x; 1.0481x over previous
"""Trainium2 Bass kernel for a 64-component mixed spherical (vMF) gaussian
distribution evaluated at 1M unit directions.

    out[s] = sum_n lambda_n * C(kappa_n) * exp(kappa_n * (dot(wi_s, mu_n) - 1))

Strategy (per core, data-parallel over S across 8 cores):
  * components n=0..63 live on SBUF/PSUM partitions; samples on the free dim
  * two half-streams of samples are packed block-diagonally so all 128
    partitions are used:  partitions 0:64  -> samples [0, S/2)
                          partitions 64:128-> samples [S/2, S)
  * TensorE:  t0[p, s] = (128/ln2) * kappa_n * dot(wi_s, mu_n).  K=18+18
    block-diag contraction (bf16 hi/lo split of A = M*kappa*mu and of wi).
    Four 512-col sample tiles run CONCURRENTLY on the four 32-row PE strips
    (tile_position=(32q, 0)).
  * The exp work is SPLIT between two engines running concurrently on
    disjoint 1024-col sample blocks (the key optimization over an all-ACT
    kernel, whose 1 elem/lane/cycle exp stream floors at ~54.6us/core):
      - ScalarE (ACT) blocks: pdf = Exp(t0 * ln2/128 + bias_n), exact
        (<=2ulp spline + bf16 out rounding), bias_n = log(lambda_n*C_n)-kappa_n.
      - VectorE (DVE) blocks: Schraudolph bit-trick exp -- one tensor_scalar:
        s = i16(max(t0 + bias_dve_n, 0)), bias_dve_n = M*bias_n + 16256 - 7.5.
        Reinterpreting s as bf16 gives 2^(s/128) piecewise-linearly
        interpolated ~ exp(y) within +-3%; the -7.5 centers the chord error.
        The i16 tile is fed to the reduction matmul bitcast as bf16.
    ~44% of sample blocks go to DVE; mixed-error l2 ~6.6e-3 (gate 2e-2).
  * TensorE:  cross-partition reduction via a sliding one-hot window,
    accumulated into a [128, 512] PSUM bank per 64-tile group; results
    rotate over the four 32-col PE strips (tile_position=(0, 32b)).
    Reduce matmuls are emitted RED_LAG blocks late so PE's strict-FIFO queue
    never blocks the next dot behind an exp-gated reduce.
  * DVE copies the accumulated bank to SBUF; DMA to HBM; host de-leaves.

History: 116 us (first correct) -> 80 (row-tiled dot) -> 74 (lagged
reductions) -> ~72 (padded DMA, warm-up) -> ~58-60 (mixed big exp blocks,
ACT-only floor) -> ACT+DVE split (this file).
"""

import math
import numpy as np
import ml_dtypes

N_COMP = 64
N_DIRS = 1048576
N_CORES = 8
S_LOCAL = N_DIRS // N_CORES      # 131072 samples per core
S_HALF = S_LOCAL // 2            # 65536 per half-stream
TILE_N = 512                     # matmul moving free dim (one PSUM bank fp32)
BLOCK = 1024                     # columns per exp instruction / psum tile
CHUNK = 16384                    # wi columns per input DMA
GROUP = 64 * TILE_N              # 32768 columns whose reductions share a bank
N_GROUPS = S_HALF // GROUP       # 2

M_SCH = 128.0 / math.log(2.0)    # Schraudolph scale: t = M*y + 16256 + DELTA
DELTA = -7.5                     # centers the piecewise-linear chord error
LN2_128 = math.log(2.0) / 128.0

# Per 32-tile chunk: 16 two-tile blocks, 'A' -> ScalarE exact exp,
# 'D' -> VectorE Schraudolph.  9A/7D balances 997ns ACT vs 1223ns DVE blocks.
PLAN = ['A', 'D', 'A', 'D', 'A', 'D', 'A', 'D',
        'A', 'D', 'A', 'D', 'A', 'D', 'A', 'A']

BF16 = ml_dtypes.bfloat16

_CACHED_NC = None


def _build_bass(repeat=1, plan=None):
    import concourse.bacc as bacc
    import concourse.tile as tile
    from concourse import mybir

    plan = plan or PLAN
    nc = bacc.Bacc("TRN2", target_bir_lowering=False, debug=False,
                   num_devices=N_CORES)

    # wi4: 512-column sample-tile t lives on partition strip 32*(t%4)+[0,18)
    # at columns [(t//4)*512, (t//4+1)*512) — four tiles are processed
    # concurrently by row-tiled matmuls on the four 32-row PE strips.
    wi4 = nc.dram_tensor("wi4", [128, S_HALF // 4], mybir.dt.bfloat16,
                         kind="ExternalInput")
    lhs_dot = nc.dram_tensor("lhs_dot", [128, 128], mybir.dt.bfloat16,
                             kind="ExternalInput")
    # Reduction weights, sliding 32-wide window: only columns 32/33 are
    # nonzero (ones over partitions [0,64) / [64,128)).  For reduce-tile j
    # (i = j%16, b = j//16) the slice big_red[:, 32-2i : 64-2i] is a
    # [128, 32] matrix whose column 2i selects the first-half sum and 2i+1
    # the second-half sum; the output goes to the 32-aligned PSUM strip
    # [32b, 32b+32).
    big_red = nc.dram_tensor("big_red", [128, 64], mybir.dt.bfloat16,
                             kind="ExternalInput")
    bias_act = nc.dram_tensor("bias_act", [128, 1], mybir.dt.float32,
                              kind="ExternalInput")
    bias_dve = nc.dram_tensor("bias_dve", [128, 1], mybir.dt.float32,
                              kind="ExternalInput")
    # raw[g, p, i]: group g, PSUM partition p = 2*j + h (reduce-tile j,
    # half-stream h), column i.  Host de-interleaves.
    out = nc.dram_tensor("out", [N_GROUPS, 128, TILE_N], mybir.dt.float32,
                         kind="ExternalOutput")

    fp32 = mybir.dt.float32
    bf16 = mybir.dt.bfloat16
    i16 = mybir.dt.int16

    with tile.TileContext(nc) as tc:
        with (
            tc.tile_pool(name="consts", bufs=1) as consts,
            tc.tile_pool(name="wi", bufs=3) as wi_pool,
            tc.tile_pool(name="pdfa", bufs=5) as pdfa_pool,
            tc.tile_pool(name="pdfd", bufs=5) as pdfd_pool,
            tc.tile_pool(name="outsb", bufs=2) as out_pool,
            tc.tile_pool(name="dot_ps", bufs=3, space="PSUM") as dot_pool,
            tc.tile_pool(name="red_ps", bufs=2, space="PSUM") as red_pool,
        ):
            lhs_dot_sb = consts.tile([128, 128], bf16)
            nc.sync.dma_start(out=lhs_dot_sb[:], in_=lhs_dot[:])
            big_red_sb = consts.tile([128, 64], bf16)
            nc.sync.dma_start(out=big_red_sb[:], in_=big_red[:])
            bias_act_sb = consts.tile([128, 1], fp32)
            nc.sync.dma_start(out=bias_act_sb[:], in_=bias_act[:])
            bias_dve_sb = consts.tile([128, 1], fp32)
            nc.sync.dma_start(out=bias_dve_sb[:], in_=bias_dve[:])

            # Dependency-free dummy exp so the ~2.7us ACT table load runs
            # at t=0, concurrent with the first DMAs/matmuls.
            warm = consts.tile([1, 8], fp32)
            nc.vector.memset(warm[:], 0.0)
            nc.scalar.activation(warm[:], warm[:],
                                 mybir.ActivationFunctionType.Exp)

            # Reduction matmuls are emitted RED_LAG blocks behind the
            # dot/exp pipeline: PE's queue is strict FIFO, so a reduce
            # waiting on an exp must not sit in front of the next dot.
            RED_LAG = 2
            pending = []        # (pdf_t, is_i16, gr, [tile idx within group])
            red_map = {}        # gr -> red accumulation psum tile

            def emit_reds(pdf_t, is_i16, gr, tiles):
                g = gr % N_GROUPS
                if gr not in red_map:
                    red_map[gr] = red_pool.tile([128, TILE_N], fp32,
                                                name="red_t", tag="red_t")
                red_t = red_map[gr]
                for mi, t in enumerate(tiles):
                    b, i = t % 4, t // 4
                    src = pdf_t[:, mi * TILE_N:(mi + 1) * TILE_N]
                    if is_i16:
                        src = src.bitcast(bf16)
                    nc.tensor.matmul(
                        red_t[32 * b:32 * b + 32, :],
                        big_red_sb[:, 32 - 2 * i:64 - 2 * i],
                        src,
                        start=(i == 0), stop=(i == 15),
                        skip_group_check=True,
                        tile_position=(0, 32 * b),
                    )
                if tiles[-1] == GROUP // TILE_N - 1:
                    out_sb = out_pool.tile([128, TILE_N], fp32)
                    nc.vector.tensor_copy(out_sb[:], red_t[:])
                    nc.sync.dma_start(out=out[g], in_=out_sb[:])
                    del red_map[gr]

            for gr in range(N_GROUPS * repeat):
                g = gr % N_GROUPS
                for ci in range(GROUP // CHUNK):
                    first = gr == 0 and ci == 0
                    if not first:
                        wi_t = wi_pool.tile([128, CHUNK // 4], bf16)
                        col0 = (g * GROUP + ci * CHUNK) // 4
                        nc.sync.dma_start(out=wi_t[:],
                                          in_=wi4[:, col0:col0 + CHUNK // 4])
                    else:
                        # Kernel warm-up: fetch the very first chunk in
                        # small pieces so the first matmuls/exps start ~4us
                        # earlier instead of waiting for one 512 KiB DMA.
                        subs = []
                        for si in range(8):
                            wi_s = wi_pool.tile([128, TILE_N], bf16,
                                                name=f"wi_first{si}",
                                                tag=f"wi_first{si}")
                            nc.sync.dma_start(
                                out=wi_s[:],
                                in_=wi4[:, si * TILE_N:(si + 1) * TILE_N])
                            subs.append(wi_s)
                    for bi in range(CHUNK // TILE_N // 2):   # 16 blocks
                        tiles_c = [2 * bi, 2 * bi + 1]
                        kind = plan[bi]
                        dot_t = dot_pool.tile([128, BLOCK], fp32,
                                              name="dot_t", tag="dot_t")
                        for mi, t_c in enumerate(tiles_c):
                            q = t_c % 4
                            if first:
                                wi_cur, u0 = subs[t_c // 4], 0
                            else:
                                wi_cur, u0 = wi_t, (t_c // 4) * TILE_N
                            nc.tensor.matmul(
                                dot_t[:, mi * TILE_N:(mi + 1) * TILE_N],
                                lhs_dot_sb[32 * q:32 * q + 18, :],
                                wi_cur[32 * q:32 * q + 18, u0:u0 + TILE_N],
                                start=True, stop=True,
                                tile_position=(32 * q, 0),
                            )
                        if kind == 'A':
                            pdf_t = pdfa_pool.tile([128, BLOCK], bf16,
                                                   name="pdfa", tag="pdfa")
                            nc.scalar.activation(
                                pdf_t[:], dot_t[:],
                                mybir.ActivationFunctionType.Exp,
                                bias=bias_act_sb[:, 0:1], scale=LN2_128,
                            )
                            is_i16 = False
                        else:
                            pdf_t = pdfd_pool.tile([128, BLOCK], i16,
                                                   name="pdfd", tag="pdfd")
                            nc.vector.tensor_scalar(
                                pdf_t[:], dot_t[:],
                                bias_dve_sb[:, 0:1], 0.0,
                                mybir.AluOpType.add, mybir.AluOpType.max,
                            )
                            is_i16 = True
                        base = ci * (CHUNK // TILE_N)
                        pending.append(
                            (pdf_t, is_i16, gr, [base + t for t in tiles_c]))
                        if len(pending) > RED_LAG:
                            emit_reds(*pending.pop(0))
            while pending:
                emit_reds(*pending.pop(0))

    nc.compile()
    return nc


def _get_nc(repeat=1):
    global _CACHED_NC
    if repeat != 1:
        return _build_bass(repeat=repeat)
    if _CACHED_NC is None:
        _CACHED_NC = _build_bass()
    return _CACHED_NC


def _host_prep(lambdas, kappas, thetas, phis, wi):
    """Build per-core input maps (tiny O(64) parameter math + bf16 hi/lo
    split and layout of wi)."""
    lambdas = np.asarray(lambdas, np.float32)
    kappas = np.asarray(kappas, np.float32)
    thetas = np.asarray(thetas, np.float32)
    phis = np.asarray(phis, np.float32)
    wi = np.ascontiguousarray(np.asarray(wi, np.float32))

    # spherical -> cartesian mean directions, scaled by M*kappa so the
    # PSUM dot lands in Schraudolph t-units (ACT undoes with scale=ln2/128)
    st = np.sin(thetas)
    mu = np.stack([st * np.cos(phis), st * np.sin(phis), np.cos(thetas)],
                  axis=-1).astype(np.float32)          # [64, 3]
    A = (mu * (kappas * np.float32(M_SCH))[:, None]).astype(np.float32)
    A1 = A.astype(BF16)
    A2 = (A - A1.astype(np.float32)).astype(BF16)

    # vMF normalization (mirrors reference._vmf_norm, fp32)
    k = np.maximum(kappas, np.float32(1e-8))
    with np.errstate(divide="ignore", over="ignore", invalid="ignore"):
        norm_k = np.where(
            kappas < np.float32(1e-5),
            np.float32(1.0 / (4.0 * math.pi)),
            k * np.float32(1.0 / (2.0 * math.pi))
            / (np.float32(1.0) - np.exp(-2.0 * k).astype(np.float32)),
        ).astype(np.float32)
    bias64 = (np.log(lambdas * norm_k) - kappas).astype(np.float32)   # [64]
    bias128 = np.concatenate([bias64, bias64]).astype(np.float32)
    bias_act = bias128.reshape(128, 1)
    bias_dve = (bias128.astype(np.float64) * M_SCH + 16256.0
                + DELTA).astype(np.float32).reshape(128, 1)

    # lhs for the dot matmul: block-diagonal bf16 hi/lo split of A
    # pairing rows: (A1,B1) (A1,B2) (A2,B1) over the 3 dims each;
    # replicated on the four 32-row PE strips for row-tiled matmuls
    A9 = np.concatenate([A1.T, A1.T, A2.T], axis=0)    # [9, 64] bf16
    lhs18 = np.zeros((18, 128), BF16)
    lhs18[0:9, 0:64] = A9
    lhs18[9:18, 64:128] = A9
    lhs_dot = np.zeros((128, 128), BF16)
    for q in range(4):
        lhs_dot[32 * q:32 * q + 18, :] = lhs18

    # lhs for the reduction matmul: sliding-window one-hot block
    big_red = np.zeros((128, 64), BF16)
    big_red[0:64, 32] = BF16(1.0)
    big_red[64:128, 33] = BF16(1.0)

    # wi bf16 hi/lo split, paired to match lhs rows
    B1 = wi.astype(BF16)                               # [S, 3]
    B2 = (wi - B1.astype(np.float32)).astype(BF16)
    B9 = np.concatenate([B1.T, B2.T, B1.T], axis=0)    # [9, S] bf16

    in_maps = []
    for c in range(N_CORES):
        c0 = c * S_LOCAL
        wi18 = np.empty((18, S_HALF), BF16)
        wi18[0:9] = B9[:, c0:c0 + S_HALF]
        wi18[9:18] = B9[:, c0 + S_HALF:c0 + S_LOCAL]
        # scatter 512-col sample tiles over the four PE row strips
        arr = wi18.reshape(18, S_HALF // TILE_N, TILE_N)
        wi4 = np.zeros((128, S_HALF // 4), BF16)
        for q in range(4):
            wi4[32 * q:32 * q + 18] = arr[:, q::4, :].reshape(18, S_HALF // 4)
        in_maps.append({
            "wi4": wi4,
            "lhs_dot": lhs_dot,
            "big_red": big_red,
            "bias_act": bias_act,
            "bias_dve": bias_dve,
        })
    return in_maps


def _assemble(results):
    out = np.empty(N_DIRS, np.float32)
    for c in range(N_CORES):
        r = np.asarray(results[c]["out"], np.float32)   # [N_GROUPS, 128, 512]
        # PSUM partition p = 32*b + 2*i + h for sample tile t = 4*i + b,
        # half-stream h
        r = r.reshape(N_GROUPS, 4, 16, 2, TILE_N)
        c0 = c * S_LOCAL
        out[c0:c0 + S_HALF] = \
            r[:, :, :, 0, :].transpose(0, 2, 1, 3).reshape(S_HALF)
        out[c0 + S_HALF:c0 + S_LOCAL] = \
            r[:, :, :, 1, :].transpose(0, 2, 1, 3).reshape(S_HALF)
    return out


def kernel(**inputs):
    from concourse.bass_utils import run_bass_kernel_spmd

    in_maps = _host_prep(**inputs)
    nc = _get_nc()
    try:
        res = run_bass_kernel_spmd(nc, in_maps, core_ids=list(range(N_CORES)))
    except Exception:
        # one retry for transient device/terminal hiccups
        res = run_bass_kernel_spmd(nc, in_maps, core_ids=list(range(N_CORES)))
    return _assemble(res.results)


def kernel_traced(**inputs):
    """Like kernel() but with NTFF tracing; returns (out, BassKernelResults)."""
    from concourse.bass_utils import run_bass_kernel_spmd

    in_maps = _host_prep(**inputs)
    nc = _get_nc()
    res = run_bass_kernel_spmd(nc, in_maps, core_ids=list(range(N_CORES)),
                               trace=True)
    return _assemble(res.results), res


# revision 3
# speedup vs baseline: 1.0912x; 1.0411x over previous
"""Trainium2 Bass kernel for a 64-component mixed spherical (vMF) gaussian
distribution evaluated at 1M unit directions.

    out[s] = sum_n lambda_n * C(kappa_n) * exp(kappa_n * (dot(wi_s, mu_n) - 1))

Strategy (per core, data-parallel over S across 8 cores):
  * components n=0..63 live on SBUF/PSUM partitions; samples on the free dim
  * two half-streams of samples are packed block-diagonally so all 128
    partitions are used:  partitions 0:64  -> samples [0, S/2)
                          partitions 64:128-> samples [S/2, S)
  * TensorE:  t0[p, s] = (128/ln2) * kappa_n * dot(wi_s, mu_n).  K=18+18
    block-diag contraction (bf16 hi/lo split of A = M*kappa*mu and of wi).
    Four 512-col sample tiles run CONCURRENTLY on the four 32-row PE strips
    (tile_position=(32q, 0)).
  * The exp work is SPLIT between two engines running concurrently on
    disjoint 1024-col sample blocks (the key optimization over an all-ACT
    kernel, whose 1 elem/lane/cycle exp stream floors at ~54.6us/core):
      - ScalarE (ACT) blocks: pdf = Exp(t0 * ln2/128 + bias_n), exact
        (<=2ulp spline + bf16 out rounding), bias_n = log(lambda_n*C_n)-kappa_n.
      - VectorE (DVE) blocks: Schraudolph bit-trick exp -- one tensor_scalar:
        s = i16(max(t0 + bias_dve_n, 0)), bias_dve_n = M*bias_n + 16256 - 7.5.
        Reinterpreting s as bf16 gives 2^(s/128) piecewise-linearly
        interpolated ~ exp(y) within +-3%; the -7.5 centers the chord error.
        The i16 tile is fed to the reduction matmul bitcast as bf16.
    ~44% of sample blocks go to DVE; mixed-error l2 ~6.6e-3 (gate 2e-2).
  * TensorE:  cross-partition reduction via a sliding one-hot window,
    accumulated into a [128, 512] PSUM bank per 64-tile group; results
    rotate over the four 32-col PE strips (tile_position=(0, 32b)).
    Reduce matmuls are emitted RED_LAG blocks late so PE's strict-FIFO queue
    never blocks the next dot behind an exp-gated reduce.
  * DVE copies the accumulated bank to SBUF; DMA to HBM; host de-leaves.

History: 116 us (first correct) -> 80 (row-tiled dot) -> 74 (lagged
reductions) -> ~72 (padded DMA, warm-up) -> ~58-60 (mixed big exp blocks,
ACT-only floor) -> ACT+DVE split (this file).
"""

import math
import numpy as np
import ml_dtypes

N_COMP = 64
N_DIRS = 1048576
N_CORES = 8
S_LOCAL = N_DIRS // N_CORES      # 131072 samples per core
S_HALF = S_LOCAL // 2            # 65536 per half-stream
TILE_N = 512                     # matmul moving free dim (one PSUM bank fp32)
BLOCK = 1024                     # columns per exp instruction / psum tile
CHUNK = 16384                    # wi columns per input DMA
GROUP = 64 * TILE_N              # 32768 columns whose reductions share a bank
N_GROUPS = S_HALF // GROUP       # 2

M_SCH = 128.0 / math.log(2.0)    # Schraudolph scale: t = M*y + 16256 + DELTA
DELTA = -7.5                     # centers the piecewise-linear chord error
LN2_128 = math.log(2.0) / 128.0

# Per 32-tile chunk: 16 two-tile blocks, 'A' -> ScalarE exact exp,
# 'D' -> VectorE Schraudolph.  11A/5D balances 997ns ACT blocks against
# 2118ns DVE blocks (DVE pays a non-overlappable pipe-flush DRAIN equal to
# ~its own duration after every op — HW-measured, see dve_bench.py).
PLAN = ['A', 'D', 'A', 'A', 'D', 'A', 'A', 'D',
        'A', 'A', 'D', 'A', 'A', 'D', 'A', 'A']

BF16 = ml_dtypes.bfloat16

_CACHED_NC = None


def _build_bass(repeat=1, plan=None):
    import concourse.bacc as bacc
    import concourse.tile as tile
    from concourse import mybir

    plan = plan or PLAN
    nc = bacc.Bacc("TRN2", target_bir_lowering=False, debug=False,
                   num_devices=N_CORES)

    # wi4: 512-column sample-tile t lives on partition strip 32*(t%4)+[0,18)
    # at columns [(t//4)*512, (t//4+1)*512) — four tiles are processed
    # concurrently by row-tiled matmuls on the four 32-row PE strips.
    wi4 = nc.dram_tensor("wi4", [128, S_HALF // 4], mybir.dt.bfloat16,
                         kind="ExternalInput")
    lhs_dot = nc.dram_tensor("lhs_dot", [128, 128], mybir.dt.bfloat16,
                             kind="ExternalInput")
    # Reduction weights, sliding 32-wide window: only columns 32/33 are
    # nonzero (ones over partitions [0,64) / [64,128)).  For reduce-tile j
    # (i = j%16, b = j//16) the slice big_red[:, 32-2i : 64-2i] is a
    # [128, 32] matrix whose column 2i selects the first-half sum and 2i+1
    # the second-half sum; the output goes to the 32-aligned PSUM strip
    # [32b, 32b+32).
    big_red = nc.dram_tensor("big_red", [128, 64], mybir.dt.bfloat16,
                             kind="ExternalInput")
    bias_act = nc.dram_tensor("bias_act", [128, 1], mybir.dt.float32,
                              kind="ExternalInput")
    bias_dve = nc.dram_tensor("bias_dve", [128, 1], mybir.dt.float32,
                              kind="ExternalInput")
    # raw[g, p, i]: group g, PSUM partition p = 2*j + h (reduce-tile j,
    # half-stream h), column i.  Host de-interleaves.
    out = nc.dram_tensor("out", [N_GROUPS, 128, TILE_N], mybir.dt.float32,
                         kind="ExternalOutput")

    fp32 = mybir.dt.float32
    bf16 = mybir.dt.bfloat16
    i16 = mybir.dt.int16

    with tile.TileContext(nc) as tc:
        with (
            tc.tile_pool(name="consts", bufs=1) as consts,
            tc.tile_pool(name="wi", bufs=3) as wi_pool,
            tc.tile_pool(name="pdfa", bufs=5) as pdfa_pool,
            tc.tile_pool(name="pdfd", bufs=5) as pdfd_pool,
            tc.tile_pool(name="outsb", bufs=2) as out_pool,
            tc.tile_pool(name="dot_ps", bufs=3, space="PSUM") as dot_pool,
            tc.tile_pool(name="red_ps", bufs=2, space="PSUM") as red_pool,
        ):
            lhs_dot_sb = consts.tile([128, 128], bf16)
            nc.sync.dma_start(out=lhs_dot_sb[:], in_=lhs_dot[:])
            big_red_sb = consts.tile([128, 64], bf16)
            nc.sync.dma_start(out=big_red_sb[:], in_=big_red[:])
            bias_act_sb = consts.tile([128, 1], fp32)
            nc.sync.dma_start(out=bias_act_sb[:], in_=bias_act[:])
            bias_dve_sb = consts.tile([128, 1], fp32)
            nc.sync.dma_start(out=bias_dve_sb[:], in_=bias_dve[:])

            # Dependency-free dummy exp so the ~2.7us ACT table load runs
            # at t=0, concurrent with the first DMAs/matmuls.
            warm = consts.tile([1, 8], fp32)
            nc.vector.memset(warm[:], 0.0)
            nc.scalar.activation(warm[:], warm[:],
                                 mybir.ActivationFunctionType.Exp)

            # Reduction matmuls are emitted RED_LAG blocks behind the
            # dot/exp pipeline: PE's queue is strict FIFO, so a reduce
            # waiting on an exp must not sit in front of the next dot.
            RED_LAG = 2
            pending = []        # (pdf_t, is_i16, gr, [tile idx within group])
            red_map = {}        # gr -> red accumulation psum tile

            def emit_reds(pdf_t, is_i16, gr, tiles):
                g = gr % N_GROUPS
                if gr not in red_map:
                    red_map[gr] = red_pool.tile([128, TILE_N], fp32,
                                                name="red_t", tag="red_t")
                red_t = red_map[gr]
                for mi, t in enumerate(tiles):
                    b, i = t % 4, t // 4
                    src = pdf_t[:, mi * TILE_N:(mi + 1) * TILE_N]
                    if is_i16:
                        src = src.bitcast(bf16)
                    nc.tensor.matmul(
                        red_t[32 * b:32 * b + 32, :],
                        big_red_sb[:, 32 - 2 * i:64 - 2 * i],
                        src,
                        start=(i == 0), stop=(i == 15),
                        skip_group_check=True,
                        tile_position=(0, 32 * b),
                    )
                if tiles[-1] == GROUP // TILE_N - 1:
                    out_sb = out_pool.tile([128, TILE_N], fp32)
                    nc.vector.tensor_copy(out_sb[:], red_t[:])
                    nc.sync.dma_start(out=out[g], in_=out_sb[:])
                    del red_map[gr]

            for gr in range(N_GROUPS * repeat):
                g = gr % N_GROUPS
                for ci in range(GROUP // CHUNK):
                    first = gr == 0 and ci == 0
                    if not first:
                        wi_t = wi_pool.tile([128, CHUNK // 4], bf16)
                        col0 = (g * GROUP + ci * CHUNK) // 4
                        nc.sync.dma_start(out=wi_t[:],
                                          in_=wi4[:, col0:col0 + CHUNK // 4])
                    else:
                        # Kernel warm-up: fetch the very first chunk in
                        # small pieces so the first matmuls/exps start ~4us
                        # earlier instead of waiting for one 512 KiB DMA.
                        subs = []
                        for si in range(8):
                            wi_s = wi_pool.tile([128, TILE_N], bf16,
                                                name=f"wi_first{si}",
                                                tag=f"wi_first{si}")
                            nc.sync.dma_start(
                                out=wi_s[:],
                                in_=wi4[:, si * TILE_N:(si + 1) * TILE_N])
                            subs.append(wi_s)
                    for bi in range(CHUNK // TILE_N // 2):   # 16 blocks
                        tiles_c = [2 * bi, 2 * bi + 1]
                        kind = plan[bi]
                        dot_t = dot_pool.tile([128, BLOCK], fp32,
                                              name="dot_t", tag="dot_t")
                        for mi, t_c in enumerate(tiles_c):
                            q = t_c % 4
                            if first:
                                wi_cur, u0 = subs[t_c // 4], 0
                            else:
                                wi_cur, u0 = wi_t, (t_c // 4) * TILE_N
                            nc.tensor.matmul(
                                dot_t[:, mi * TILE_N:(mi + 1) * TILE_N],
                                lhs_dot_sb[32 * q:32 * q + 18, :],
                                wi_cur[32 * q:32 * q + 18, u0:u0 + TILE_N],
                                start=True, stop=True,
                                tile_position=(32 * q, 0),
                            )
                        if kind == 'A':
                            pdf_t = pdfa_pool.tile([128, BLOCK], bf16,
                                                   name="pdfa", tag="pdfa")
                            nc.scalar.activation(
                                pdf_t[:], dot_t[:],
                                mybir.ActivationFunctionType.Exp,
                                bias=bias_act_sb[:, 0:1], scale=LN2_128,
                            )
                            is_i16 = False
                        else:
                            pdf_t = pdfd_pool.tile([128, BLOCK], i16,
                                                   name="pdfd", tag="pdfd")
                            nc.vector.tensor_scalar(
                                pdf_t[:], dot_t[:],
                                bias_dve_sb[:, 0:1], 0.0,
                                mybir.AluOpType.add, mybir.AluOpType.max,
                            )
                            is_i16 = True
                        base = ci * (CHUNK // TILE_N)
                        pending.append(
                            (pdf_t, is_i16, gr, [base + t for t in tiles_c]))
                        if len(pending) > RED_LAG:
                            emit_reds(*pending.pop(0))
            while pending:
                emit_reds(*pending.pop(0))

    nc.compile()
    return nc


def _get_nc(repeat=1):
    global _CACHED_NC
    if repeat != 1:
        return _build_bass(repeat=repeat)
    if _CACHED_NC is None:
        _CACHED_NC = _build_bass()
    return _CACHED_NC


def _host_prep(lambdas, kappas, thetas, phis, wi):
    """Build per-core input maps (tiny O(64) parameter math + bf16 hi/lo
    split and layout of wi)."""
    lambdas = np.asarray(lambdas, np.float32)
    kappas = np.asarray(kappas, np.float32)
    thetas = np.asarray(thetas, np.float32)
    phis = np.asarray(phis, np.float32)
    wi = np.ascontiguousarray(np.asarray(wi, np.float32))

    # spherical -> cartesian mean directions, scaled by M*kappa so the
    # PSUM dot lands in Schraudolph t-units (ACT undoes with scale=ln2/128)
    st = np.sin(thetas)
    mu = np.stack([st * np.cos(phis), st * np.sin(phis), np.cos(thetas)],
                  axis=-1).astype(np.float32)          # [64, 3]
    A = (mu * (kappas * np.float32(M_SCH))[:, None]).astype(np.float32)
    A1 = A.astype(BF16)
    A2 = (A - A1.astype(np.float32)).astype(BF16)

    # vMF normalization (mirrors reference._vmf_norm, fp32)
    k = np.maximum(kappas, np.float32(1e-8))
    with np.errstate(divide="ignore", over="ignore", invalid="ignore"):
        norm_k = np.where(
            kappas < np.float32(1e-5),
            np.float32(1.0 / (4.0 * math.pi)),
            k * np.float32(1.0 / (2.0 * math.pi))
            / (np.float32(1.0) - np.exp(-2.0 * k).astype(np.float32)),
        ).astype(np.float32)
    bias64 = (np.log(lambdas * norm_k) - kappas).astype(np.float32)   # [64]
    bias128 = np.concatenate([bias64, bias64]).astype(np.float32)
    bias_act = bias128.reshape(128, 1)
    bias_dve = (bias128.astype(np.float64) * M_SCH + 16256.0
                + DELTA).astype(np.float32).reshape(128, 1)

    # lhs for the dot matmul: block-diagonal bf16 hi/lo split of A
    # pairing rows: (A1,B1) (A1,B2) (A2,B1) over the 3 dims each;
    # replicated on the four 32-row PE strips for row-tiled matmuls
    A9 = np.concatenate([A1.T, A1.T, A2.T], axis=0)    # [9, 64] bf16
    lhs18 = np.zeros((18, 128), BF16)
    lhs18[0:9, 0:64] = A9
    lhs18[9:18, 64:128] = A9
    lhs_dot = np.zeros((128, 128), BF16)
    for q in range(4):
        lhs_dot[32 * q:32 * q + 18, :] = lhs18

    # lhs for the reduction matmul: sliding-window one-hot block
    big_red = np.zeros((128, 64), BF16)
    big_red[0:64, 32] = BF16(1.0)
    big_red[64:128, 33] = BF16(1.0)

    # wi bf16 hi/lo split, paired to match lhs rows
    B1 = wi.astype(BF16)                               # [S, 3]
    B2 = (wi - B1.astype(np.float32)).astype(BF16)
    B9 = np.concatenate([B1.T, B2.T, B1.T], axis=0)    # [9, S] bf16

    in_maps = []
    for c in range(N_CORES):
        c0 = c * S_LOCAL
        wi18 = np.empty((18, S_HALF), BF16)
        wi18[0:9] = B9[:, c0:c0 + S_HALF]
        wi18[9:18] = B9[:, c0 + S_HALF:c0 + S_LOCAL]
        # scatter 512-col sample tiles over the four PE row strips
        arr = wi18.reshape(18, S_HALF // TILE_N, TILE_N)
        wi4 = np.zeros((128, S_HALF // 4), BF16)
        for q in range(4):
            wi4[32 * q:32 * q + 18] = arr[:, q::4, :].reshape(18, S_HALF // 4)
        in_maps.append({
            "wi4": wi4,
            "lhs_dot": lhs_dot,
            "big_red": big_red,
            "bias_act": bias_act,
            "bias_dve": bias_dve,
        })
    return in_maps


def _assemble(results):
    out = np.empty(N_DIRS, np.float32)
    for c in range(N_CORES):
        r = np.asarray(results[c]["out"], np.float32)   # [N_GROUPS, 128, 512]
        # PSUM partition p = 32*b + 2*i + h for sample tile t = 4*i + b,
        # half-stream h
        r = r.reshape(N_GROUPS, 4, 16, 2, TILE_N)
        c0 = c * S_LOCAL
        out[c0:c0 + S_HALF] = \
            r[:, :, :, 0, :].transpose(0, 2, 1, 3).reshape(S_HALF)
        out[c0 + S_HALF:c0 + S_LOCAL] = \
            r[:, :, :, 1, :].transpose(0, 2, 1, 3).reshape(S_HALF)
    return out


def kernel(**inputs):
    from concourse.bass_utils import run_bass_kernel_spmd

    in_maps = _host_prep(**inputs)
    nc = _get_nc()
    try:
        res = run_bass_kernel_spmd(nc, in_maps, core_ids=list(range(N_CORES)))
    except Exception:
        # one retry for transient device/terminal hiccups
        res = run_bass_kernel_spmd(nc, in_maps, core_ids=list(range(N_CORES)))
    return _assemble(res.results)


def kernel_traced(**inputs):
    """Like kernel() but with NTFF tracing; returns (out, BassKernelResults)."""
    from concourse.bass_utils import run_bass_kernel_spmd

    in_maps = _host_prep(**inputs)
    nc = _get_nc()
    res = run_bass_kernel_spmd(nc, in_maps, core_ids=list(range(N_CORES)),
                               trace=True)
    return _assemble(res.results), res


# revision 5
# speedup vs baseline: 1.1479x; 1.0520x over previous
"""Trainium2 Bass kernel for a 64-component mixed spherical (vMF) gaussian
distribution evaluated at 1M unit directions.

    out[s] = sum_n lambda_n * C(kappa_n) * exp(kappa_n * (dot(wi_s, mu_n) - 1))

Strategy (per core, data-parallel over S across 8 cores):
  * components n=0..63 live on SBUF/PSUM partitions; samples on the free dim
  * two half-streams of samples are packed block-diagonally so all 128
    partitions are used:  partitions 0:64  -> samples [0, S/2)
                          partitions 64:128-> samples [S/2, S)
  * TensorE:  t0[p, s] = (128/ln2) * kappa_n * dot(wi_s, mu_n).  K=18+18
    block-diag contraction (bf16 hi/lo split of A = M*kappa*mu and of wi).
    Four 512-col sample tiles run CONCURRENTLY on the four 32-row PE strips
    (tile_position=(32q, 0)).
  * The exp work is SPLIT between two engines running concurrently on
    disjoint 1024-col sample blocks (the key optimization over an all-ACT
    kernel, whose 1 elem/lane/cycle exp stream floors at ~54.6us/core):
      - ScalarE (ACT) blocks: pdf = Exp(t0 * ln2/128 + bias_n), exact
        (<=2ulp spline + bf16 out rounding), bias_n = log(lambda_n*C_n)-kappa_n.
      - VectorE (DVE) blocks: Schraudolph bit-trick exp -- one tensor_scalar:
        s = i16(max(t0 + bias_dve_n, 0)), bias_dve_n = M*bias_n + 16256 - 7.5.
        Reinterpreting s as bf16 gives 2^(s/128) piecewise-linearly
        interpolated ~ exp(y) within +-3%; the -7.5 centers the chord error.
        The i16 tile is fed to the reduction matmul bitcast as bf16.
    ~44% of sample blocks go to DVE; mixed-error l2 ~6.6e-3 (gate 2e-2).
  * TensorE:  cross-partition reduction via a sliding one-hot window,
    accumulated into a [128, 512] PSUM bank per 64-tile group; results
    rotate over the four 32-col PE strips (tile_position=(0, 32b)).
    Reduce matmuls are emitted RED_LAG blocks late so PE's strict-FIFO queue
    never blocks the next dot behind an exp-gated reduce.
  * DVE copies the accumulated bank to SBUF; DMA to HBM; host de-leaves.

History: 116 us (first correct) -> 80 (row-tiled dot) -> 74 (lagged
reductions) -> ~72 (padded DMA, warm-up) -> ~58-60 (mixed big exp blocks,
ACT-only floor) -> ACT+DVE split (this file).
"""

import math
import numpy as np
import ml_dtypes

N_COMP = 64
N_DIRS = 1048576
N_CORES = 8
S_LOCAL = N_DIRS // N_CORES      # 131072 samples per core
S_HALF = S_LOCAL // 2            # 65536 per half-stream
TILE_N = 512                     # matmul moving free dim (one PSUM bank fp32)
BLOCK = 1024                     # columns per exp instruction / psum tile
CHUNK = 16384                    # wi columns per input DMA
GROUP = 64 * TILE_N              # 32768 columns whose reductions share a bank
N_GROUPS = S_HALF // GROUP       # 2

M_SCH = 128.0 / math.log(2.0)    # Schraudolph scale: t = M*y + 16256 + DELTA
DELTA = -7.5                     # centers the piecewise-linear chord error
LN2_128 = math.log(2.0) / 128.0

# Per 32-tile chunk: 16 two-tile blocks, 'A' -> ScalarE exact exp,
# 'D' -> VectorE Schraudolph.  11A/5D balances 997ns ACT blocks against
# 2118ns DVE blocks (DVE pays a non-overlappable pipe-flush DRAIN equal to
# ~its own duration after every op — HW-measured, see dve_bench.py).
PLAN = ['A', 'D', 'A', 'A', 'D', 'A', 'A', 'D',
        'A', 'A', 'D', 'A', 'A', 'D', 'A', 'A']

BF16 = ml_dtypes.bfloat16

_CACHED_NC = None


def _build_bass(repeat=1, plan=None):
    import concourse.bacc as bacc
    import concourse.tile as tile
    from concourse import mybir

    plan = plan or PLAN
    nc = bacc.Bacc("TRN2", target_bir_lowering=False, debug=False,
                   num_devices=N_CORES)

    # wi4: 512-column sample-tile t lives on partition strip 32*(t%4)+[0,18)
    # at columns [(t//4)*512, (t//4+1)*512) — four tiles are processed
    # concurrently by row-tiled matmuls on the four 32-row PE strips.
    wi4 = nc.dram_tensor("wi4", [128, S_HALF // 4], mybir.dt.bfloat16,
                         kind="ExternalInput")
    lhs_dot = nc.dram_tensor("lhs_dot", [128, 128], mybir.dt.bfloat16,
                             kind="ExternalInput")
    # Reduction weights, sliding 32-wide window: only columns 32/33 are
    # nonzero (ones over partitions [0,64) / [64,128)).  For reduce-tile j
    # (i = j%16, b = j//16) the slice big_red[:, 32-2i : 64-2i] is a
    # [128, 32] matrix whose column 2i selects the first-half sum and 2i+1
    # the second-half sum; the output goes to the 32-aligned PSUM strip
    # [32b, 32b+32).
    big_red = nc.dram_tensor("big_red", [128, 64], mybir.dt.bfloat16,
                             kind="ExternalInput")
    bias_act = nc.dram_tensor("bias_act", [128, 1], mybir.dt.float32,
                              kind="ExternalInput")
    bias_dve = nc.dram_tensor("bias_dve", [128, 1], mybir.dt.float32,
                              kind="ExternalInput")
    # raw[g, p, i]: group g, PSUM partition p = 2*j + h (reduce-tile j,
    # half-stream h), column i.  Host de-interleaves.
    out = nc.dram_tensor("out", [N_GROUPS, 128, TILE_N], mybir.dt.float32,
                         kind="ExternalOutput")

    fp32 = mybir.dt.float32
    bf16 = mybir.dt.bfloat16
    i16 = mybir.dt.int16

    with tile.TileContext(nc) as tc:
        with (
            tc.tile_pool(name="consts", bufs=1) as consts,
            tc.tile_pool(name="wi", bufs=3) as wi_pool,
            tc.tile_pool(name="pdfa", bufs=5) as pdfa_pool,
            tc.tile_pool(name="pdfd", bufs=5) as pdfd_pool,
            tc.tile_pool(name="outsb", bufs=2) as out_pool,
            tc.tile_pool(name="dot_ps", bufs=3, space="PSUM") as dot_pool,
            tc.tile_pool(name="red_ps", bufs=2, space="PSUM") as red_pool,
        ):
            lhs_dot_sb = consts.tile([128, 128], bf16)
            nc.sync.dma_start(out=lhs_dot_sb[:], in_=lhs_dot[:])
            big_red_sb = consts.tile([128, 64], bf16)
            nc.sync.dma_start(out=big_red_sb[:], in_=big_red[:])
            bias_act_sb = consts.tile([128, 1], fp32)
            nc.sync.dma_start(out=bias_act_sb[:], in_=bias_act[:])
            bias_dve_sb = consts.tile([128, 1], fp32)
            nc.sync.dma_start(out=bias_dve_sb[:], in_=bias_dve[:])

            # Dependency-free dummy exp so the ~2.7us ACT table load runs
            # at t=0, concurrent with the first DMAs/matmuls.
            warm = consts.tile([1, 8], fp32)
            nc.vector.memset(warm[:], 0.0)
            nc.scalar.activation(warm[:], warm[:],
                                 mybir.ActivationFunctionType.Exp)

            # Reduction matmuls are emitted RED_LAG blocks behind the
            # dot/exp pipeline: PE's queue is strict FIFO, so a reduce
            # waiting on an exp must not sit in front of the next dot.
            RED_LAG = 4
            pending = []        # (pdf_t, is_i16, gr, [tile idx within group])
            red_map = {}        # gr -> red accumulation psum tile

            def emit_reds(pdf_t, is_i16, gr, tiles):
                g = gr % N_GROUPS
                if gr not in red_map:
                    red_map[gr] = red_pool.tile([128, TILE_N], fp32,
                                                name="red_t", tag="red_t")
                red_t = red_map[gr]
                for mi, t in enumerate(tiles):
                    b, i = t % 4, t // 4
                    src = pdf_t[:, mi * TILE_N:(mi + 1) * TILE_N]
                    if is_i16:
                        src = src.bitcast(bf16)
                    nc.tensor.matmul(
                        red_t[32 * b:32 * b + 32, :],
                        big_red_sb[:, 32 - 2 * i:64 - 2 * i],
                        src,
                        start=(i == 0), stop=(i == 15),
                        skip_group_check=True,
                        tile_position=(0, 32 * b),
                    )
                if tiles[-1] == GROUP // TILE_N - 1:
                    out_sb = out_pool.tile([128, TILE_N], fp32)
                    nc.scalar.copy(out_sb[:], red_t[:])
                    nc.sync.dma_start(out=out[g], in_=out_sb[:])
                    del red_map[gr]

            for gr in range(N_GROUPS * repeat):
                g = gr % N_GROUPS
                for ci in range(GROUP // CHUNK):
                    first = gr == 0 and ci == 0
                    if not first:
                        wi_t = wi_pool.tile([128, CHUNK // 4], bf16)
                        col0 = (g * GROUP + ci * CHUNK) // 4
                        nc.sync.dma_start(out=wi_t[:],
                                          in_=wi4[:, col0:col0 + CHUNK // 4])
                    else:
                        # Kernel warm-up: fetch the very first chunk in
                        # small pieces so the first matmuls/exps start ~4us
                        # earlier instead of waiting for one 512 KiB DMA.
                        subs = []
                        for si in range(8):
                            wi_s = wi_pool.tile([128, TILE_N], bf16,
                                                name=f"wi_first{si}",
                                                tag=f"wi_first{si}")
                            nc.sync.dma_start(
                                out=wi_s[:],
                                in_=wi4[:, si * TILE_N:(si + 1) * TILE_N])
                            subs.append(wi_s)
                    for bi in range(CHUNK // TILE_N // 2):   # 16 blocks
                        tiles_c = [2 * bi, 2 * bi + 1]
                        kind = plan[bi]
                        dot_t = dot_pool.tile([128, BLOCK], fp32,
                                              name="dot_t", tag="dot_t")
                        for mi, t_c in enumerate(tiles_c):
                            q = t_c % 4
                            if first:
                                wi_cur, u0 = subs[t_c // 4], 0
                            else:
                                wi_cur, u0 = wi_t, (t_c // 4) * TILE_N
                            nc.tensor.matmul(
                                dot_t[:, mi * TILE_N:(mi + 1) * TILE_N],
                                lhs_dot_sb[32 * q:32 * q + 18, :],
                                wi_cur[32 * q:32 * q + 18, u0:u0 + TILE_N],
                                start=True, stop=True,
                                tile_position=(32 * q, 0),
                            )
                        if kind == 'A':
                            pdf_t = pdfa_pool.tile([128, BLOCK], bf16,
                                                   name="pdfa", tag="pdfa")
                            nc.scalar.activation(
                                pdf_t[:], dot_t[:],
                                mybir.ActivationFunctionType.Exp,
                                bias=bias_act_sb[:, 0:1], scale=LN2_128,
                            )
                            is_i16 = False
                        else:
                            pdf_t = pdfd_pool.tile([128, BLOCK], i16,
                                                   name="pdfd", tag="pdfd")
                            nc.vector.tensor_scalar(
                                pdf_t[:], dot_t[:],
                                bias_dve_sb[:, 0:1], 0.0,
                                mybir.AluOpType.add, mybir.AluOpType.max,
                            )
                            is_i16 = True
                        base = ci * (CHUNK // TILE_N)
                        pending.append(
                            (pdf_t, is_i16, gr, [base + t for t in tiles_c]))
                        if len(pending) > RED_LAG:
                            emit_reds(*pending.pop(0))
            while pending:
                emit_reds(*pending.pop(0))

    nc.compile()
    return nc


def _get_nc(repeat=1):
    global _CACHED_NC
    if repeat != 1:
        return _build_bass(repeat=repeat)
    if _CACHED_NC is None:
        _CACHED_NC = _build_bass()
    return _CACHED_NC


def _host_prep(lambdas, kappas, thetas, phis, wi):
    """Build per-core input maps (tiny O(64) parameter math + bf16 hi/lo
    split and layout of wi)."""
    lambdas = np.asarray(lambdas, np.float32)
    kappas = np.asarray(kappas, np.float32)
    thetas = np.asarray(thetas, np.float32)
    phis = np.asarray(phis, np.float32)
    wi = np.ascontiguousarray(np.asarray(wi, np.float32))

    # spherical -> cartesian mean directions, scaled by M*kappa so the
    # PSUM dot lands in Schraudolph t-units (ACT undoes with scale=ln2/128)
    st = np.sin(thetas)
    mu = np.stack([st * np.cos(phis), st * np.sin(phis), np.cos(thetas)],
                  axis=-1).astype(np.float32)          # [64, 3]
    A = (mu * (kappas * np.float32(M_SCH))[:, None]).astype(np.float32)
    A1 = A.astype(BF16)
    A2 = (A - A1.astype(np.float32)).astype(BF16)

    # vMF normalization (mirrors reference._vmf_norm, fp32)
    k = np.maximum(kappas, np.float32(1e-8))
    with np.errstate(divide="ignore", over="ignore", invalid="ignore"):
        norm_k = np.where(
            kappas < np.float32(1e-5),
            np.float32(1.0 / (4.0 * math.pi)),
            k * np.float32(1.0 / (2.0 * math.pi))
            / (np.float32(1.0) - np.exp(-2.0 * k).astype(np.float32)),
        ).astype(np.float32)
    bias64 = (np.log(lambdas * norm_k) - kappas).astype(np.float32)   # [64]
    bias128 = np.concatenate([bias64, bias64]).astype(np.float32)
    bias_act = bias128.reshape(128, 1)
    bias_dve = (bias128.astype(np.float64) * M_SCH + 16256.0
                + DELTA).astype(np.float32).reshape(128, 1)

    # lhs for the dot matmul: block-diagonal bf16 hi/lo split of A
    # pairing rows: (A1,B1) (A1,B2) (A2,B1) over the 3 dims each;
    # replicated on the four 32-row PE strips for row-tiled matmuls
    A9 = np.concatenate([A1.T, A1.T, A2.T], axis=0)    # [9, 64] bf16
    lhs18 = np.zeros((18, 128), BF16)
    lhs18[0:9, 0:64] = A9
    lhs18[9:18, 64:128] = A9
    lhs_dot = np.zeros((128, 128), BF16)
    for q in range(4):
        lhs_dot[32 * q:32 * q + 18, :] = lhs18

    # lhs for the reduction matmul: sliding-window one-hot block
    big_red = np.zeros((128, 64), BF16)
    big_red[0:64, 32] = BF16(1.0)
    big_red[64:128, 33] = BF16(1.0)

    # wi bf16 hi/lo split, paired to match lhs rows
    B1 = wi.astype(BF16)                               # [S, 3]
    B2 = (wi - B1.astype(np.float32)).astype(BF16)
    B9 = np.concatenate([B1.T, B2.T, B1.T], axis=0)    # [9, S] bf16

    in_maps = []
    for c in range(N_CORES):
        c0 = c * S_LOCAL
        wi18 = np.empty((18, S_HALF), BF16)
        wi18[0:9] = B9[:, c0:c0 + S_HALF]
        wi18[9:18] = B9[:, c0 + S_HALF:c0 + S_LOCAL]
        # scatter 512-col sample tiles over the four PE row strips
        arr = wi18.reshape(18, S_HALF // TILE_N, TILE_N)
        wi4 = np.zeros((128, S_HALF // 4), BF16)
        for q in range(4):
            wi4[32 * q:32 * q + 18] = arr[:, q::4, :].reshape(18, S_HALF // 4)
        in_maps.append({
            "wi4": wi4,
            "lhs_dot": lhs_dot,
            "big_red": big_red,
            "bias_act": bias_act,
            "bias_dve": bias_dve,
        })
    return in_maps


def _assemble(results):
    out = np.empty(N_DIRS, np.float32)
    for c in range(N_CORES):
        r = np.asarray(results[c]["out"], np.float32)   # [N_GROUPS, 128, 512]
        # PSUM partition p = 32*b + 2*i + h for sample tile t = 4*i + b,
        # half-stream h
        r = r.reshape(N_GROUPS, 4, 16, 2, TILE_N)
        c0 = c * S_LOCAL
        out[c0:c0 + S_HALF] = \
            r[:, :, :, 0, :].transpose(0, 2, 1, 3).reshape(S_HALF)
        out[c0 + S_HALF:c0 + S_LOCAL] = \
            r[:, :, :, 1, :].transpose(0, 2, 1, 3).reshape(S_HALF)
    return out


def kernel(**inputs):
    from concourse.bass_utils import run_bass_kernel_spmd

    in_maps = _host_prep(**inputs)
    nc = _get_nc()
    try:
        res = run_bass_kernel_spmd(nc, in_maps, core_ids=list(range(N_CORES)))
    except Exception:
        # one retry for transient device/terminal hiccups
        res = run_bass_kernel_spmd(nc, in_maps, core_ids=list(range(N_CORES)))
    return _assemble(res.results)


def kernel_traced(**inputs):
    """Like kernel() but with NTFF tracing; returns (out, BassKernelResults)."""
    from concourse.bass_utils import run_bass_kernel_spmd

    in_maps = _host_prep(**inputs)
    nc = _get_nc()
    res = run_bass_kernel_spmd(nc, in_maps, core_ids=list(range(N_CORES)),
                               trace=True)
    return _assemble(res.results), res


# revision 6
# speedup vs baseline: 1.3493x; 1.1754x over previous
"""Trainium2 Bass kernel for a 64-component mixed spherical (vMF) gaussian
distribution evaluated at 1M unit directions.

    out[s] = sum_n lambda_n * C(kappa_n) * exp(kappa_n * (dot(wi_s, mu_n) - 1))

Strategy (per core, data-parallel over S across 8 cores):
  * components n=0..63 live on SBUF/PSUM partitions; samples on the free dim
  * two half-streams of samples are packed block-diagonally so all 128
    partitions are used:  partitions 0:64  -> samples [0, S/2)
                          partitions 64:128-> samples [S/2, S)
  * TensorE:  t0[p, s] = (128/ln2) * kappa_n * dot(wi_s, mu_n).  K=18+18
    block-diag contraction (bf16 hi/lo split of A = M*kappa*mu and of wi).
    Four 512-col sample tiles run CONCURRENTLY on the four 32-row PE strips
    (tile_position=(32q, 0)).
  * The exp work is SPLIT between two engines running concurrently on
    disjoint 1024-col sample blocks (the key optimization over an all-ACT
    kernel, whose 1 elem/lane/cycle exp stream floors at ~54.6us/core):
      - ScalarE (ACT) blocks: pdf = Exp(t0 * ln2/128 + bias_n), exact
        (<=2ulp spline + bf16 out rounding), bias_n = log(lambda_n*C_n)-kappa_n.
      - VectorE (DVE) blocks: Schraudolph bit-trick exp -- one tensor_scalar:
        s = i16(max(t0 + bias_dve_n, 0)), bias_dve_n = M*bias_n + 16256 - 7.5.
        Reinterpreting s as bf16 gives 2^(s/128) piecewise-linearly
        interpolated ~ exp(y) within +-3%; the -7.5 centers the chord error.
        The i16 tile is fed to the reduction matmul bitcast as bf16.
    ~44% of sample blocks go to DVE; mixed-error l2 ~6.6e-3 (gate 2e-2).
  * TensorE:  cross-partition reduction via a sliding one-hot window,
    accumulated into a [128, 512] PSUM bank per 64-tile group; results
    rotate over the four 32-col PE strips (tile_position=(0, 32b)).
    Reduce matmuls are emitted RED_LAG blocks late so PE's strict-FIFO queue
    never blocks the next dot behind an exp-gated reduce.
  * DVE copies the accumulated bank to SBUF; DMA to HBM; host de-leaves.

History: 116 us (first correct) -> 80 (row-tiled dot) -> 74 (lagged
reductions) -> ~72 (padded DMA, warm-up) -> ~58-60 (mixed big exp blocks,
ACT-only floor) -> ACT+DVE split (this file).
"""

import math
import numpy as np
import ml_dtypes

N_COMP = 64
N_DIRS = 1048576
N_CORES = 8
S_LOCAL = N_DIRS // N_CORES      # 131072 samples per core
S_HALF = S_LOCAL // 2            # 65536 per half-stream
TILE_N = 512                     # matmul moving free dim (one PSUM bank fp32)
BLOCK = 1024                     # columns per exp instruction / psum tile
CHUNK = 16384                    # wi columns per input DMA
GROUP = 64 * TILE_N              # 32768 columns whose reductions share a bank
N_GROUPS = S_HALF // GROUP       # 2

M_SCH = 128.0 / math.log(2.0)    # Schraudolph scale: t = M*y + 16256 + DELTA
DELTA = -7.5                     # centers the piecewise-linear chord error
LN2_128 = math.log(2.0) / 128.0

# Per 32-tile chunk: 16 two-tile blocks, 'A' -> ScalarE exact exp,
# 'D' -> VectorE Schraudolph.  11A/5D balances 997ns ACT blocks against
# 2118ns DVE blocks (DVE pays a non-overlappable pipe-flush DRAIN equal to
# ~its own duration after every op — HW-measured, see dve_bench.py).
PLAN = ['A', 'D', 'A', 'D', 'A', 'A', 'D', 'A',
        'A', 'D', 'A', 'A', 'D', 'A', 'D', 'A']

BF16 = ml_dtypes.bfloat16

_CACHED_NC = None


def _build_bass(repeat=1, plan=None):
    import concourse.bacc as bacc
    import concourse.tile as tile
    from concourse import mybir

    plan = plan or PLAN
    nc = bacc.Bacc("TRN2", target_bir_lowering=False, debug=False,
                   num_devices=N_CORES)

    # wi4: 512-column sample-tile t lives on partition strip 32*(t%4)+[0,18)
    # at columns [(t//4)*512, (t//4+1)*512) — four tiles are processed
    # concurrently by row-tiled matmuls on the four 32-row PE strips.
    wi4 = nc.dram_tensor("wi4", [128, S_HALF // 4], mybir.dt.bfloat16,
                         kind="ExternalInput")
    lhs_dot = nc.dram_tensor("lhs_dot", [128, 128], mybir.dt.bfloat16,
                             kind="ExternalInput")
    # Reduction weights, sliding 32-wide window: only columns 32/33 are
    # nonzero (ones over partitions [0,64) / [64,128)).  For reduce-tile j
    # (i = j%16, b = j//16) the slice big_red[:, 32-2i : 64-2i] is a
    # [128, 32] matrix whose column 2i selects the first-half sum and 2i+1
    # the second-half sum; the output goes to the 32-aligned PSUM strip
    # [32b, 32b+32).
    big_red = nc.dram_tensor("big_red", [128, 64], mybir.dt.bfloat16,
                             kind="ExternalInput")
    bias_act = nc.dram_tensor("bias_act", [128, 1], mybir.dt.float32,
                              kind="ExternalInput")
    bias_dve = nc.dram_tensor("bias_dve", [128, 1], mybir.dt.float32,
                              kind="ExternalInput")
    # raw[g, p, i]: group g, PSUM partition p = 2*j + h (reduce-tile j,
    # half-stream h), column i.  Host de-interleaves.
    out = nc.dram_tensor("out", [N_GROUPS, 128, TILE_N], mybir.dt.float32,
                         kind="ExternalOutput")

    fp32 = mybir.dt.float32
    bf16 = mybir.dt.bfloat16
    i16 = mybir.dt.int16

    with tile.TileContext(nc) as tc:
        with (
            tc.tile_pool(name="consts", bufs=1) as consts,
            tc.tile_pool(name="wi", bufs=3) as wi_pool,
            tc.tile_pool(name="pdfa", bufs=5) as pdfa_pool,
            tc.tile_pool(name="pdfd", bufs=5) as pdfd_pool,
            tc.tile_pool(name="outsb", bufs=2) as out_pool,
            tc.tile_pool(name="dot_ps", bufs=3, space="PSUM") as dot_pool,
            tc.tile_pool(name="red_ps", bufs=2, space="PSUM") as red_pool,
        ):
            lhs_dot_sb = consts.tile([128, 128], bf16)
            nc.sync.dma_start(out=lhs_dot_sb[:], in_=lhs_dot[:])
            big_red_sb = consts.tile([128, 64], bf16)
            nc.sync.dma_start(out=big_red_sb[:], in_=big_red[:])
            bias_act_sb = consts.tile([128, 1], fp32)
            nc.sync.dma_start(out=bias_act_sb[:], in_=bias_act[:])
            bias_dve_sb = consts.tile([128, 1], fp32)
            nc.sync.dma_start(out=bias_dve_sb[:], in_=bias_dve[:])

            # Dependency-free dummy exp so the ~2.7us ACT table load runs
            # at t=0, concurrent with the first DMAs/matmuls.
            warm = consts.tile([1, 8], fp32)
            nc.vector.memset(warm[:], 0.0)
            nc.scalar.activation(warm[:], warm[:],
                                 mybir.ActivationFunctionType.Exp)

            # Reduction matmuls are emitted RED_LAG blocks behind the
            # dot/exp pipeline: PE's queue is strict FIFO, so a reduce
            # waiting on an exp must not sit in front of the next dot.
            RED_LAG = 4
            pending = []        # (pdf_t, is_i16, gr, [tile idx within group])
            red_map = {}        # gr -> red accumulation psum tile

            def emit_reds(pdf_t, is_i16, gr, tiles):
                g = gr % N_GROUPS
                if gr not in red_map:
                    red_map[gr] = red_pool.tile([128, TILE_N], fp32,
                                                name="red_t", tag="red_t")
                red_t = red_map[gr]
                for mi, t in enumerate(tiles):
                    b, i = t % 4, t // 4
                    src = pdf_t[:, mi * TILE_N:(mi + 1) * TILE_N]
                    if is_i16:
                        src = src.bitcast(bf16)
                    nc.tensor.matmul(
                        red_t[32 * b:32 * b + 32, :],
                        big_red_sb[:, 32 - 2 * i:64 - 2 * i],
                        src,
                        start=(i == 0), stop=(i == 15),
                        skip_group_check=True,
                        tile_position=(0, 32 * b),
                    )
                if tiles[-1] == GROUP // TILE_N - 1:
                    out_sb = out_pool.tile([128, TILE_N], fp32)
                    nc.scalar.copy(out_sb[:], red_t[:])
                    nc.sync.dma_start(out=out[g], in_=out_sb[:])
                    del red_map[gr]

            for gr in range(N_GROUPS * repeat):
                g = gr % N_GROUPS
                for ci in range(GROUP // CHUNK):
                    first = gr == 0 and ci == 0
                    if not first:
                        wi_t = wi_pool.tile([128, CHUNK // 4], bf16)
                        col0 = (g * GROUP + ci * CHUNK) // 4
                        nc.sync.dma_start(out=wi_t[:],
                                          in_=wi4[:, col0:col0 + CHUNK // 4])
                    else:
                        # Kernel warm-up: fetch the very first chunk in
                        # small pieces so the first matmuls/exps start ~4us
                        # earlier instead of waiting for one 512 KiB DMA.
                        subs = []
                        for si in range(8):
                            wi_s = wi_pool.tile([128, TILE_N], bf16,
                                                name=f"wi_first{si}",
                                                tag=f"wi_first{si}")
                            nc.sync.dma_start(
                                out=wi_s[:],
                                in_=wi4[:, si * TILE_N:(si + 1) * TILE_N])
                            subs.append(wi_s)
                    for bi in range(CHUNK // TILE_N // 2):   # 16 blocks
                        tiles_c = [2 * bi, 2 * bi + 1]
                        kind = plan[bi]
                        dot_t = dot_pool.tile([128, BLOCK], fp32,
                                              name="dot_t", tag="dot_t")
                        for mi, t_c in enumerate(tiles_c):
                            q = t_c % 4
                            if first:
                                wi_cur, u0 = subs[t_c // 4], 0
                            else:
                                wi_cur, u0 = wi_t, (t_c // 4) * TILE_N
                            nc.tensor.matmul(
                                dot_t[:, mi * TILE_N:(mi + 1) * TILE_N],
                                lhs_dot_sb[32 * q:32 * q + 18, :],
                                wi_cur[32 * q:32 * q + 18, u0:u0 + TILE_N],
                                start=True, stop=True,
                                tile_position=(32 * q, 0),
                            )
                        if kind == 'A':
                            pdf_t = pdfa_pool.tile([128, BLOCK], bf16,
                                                   name="pdfa", tag="pdfa")
                            nc.scalar.activation(
                                pdf_t[:], dot_t[:],
                                mybir.ActivationFunctionType.Exp,
                                bias=bias_act_sb[:, 0:1], scale=LN2_128,
                            )
                            is_i16 = False
                        else:
                            pdf_t = pdfd_pool.tile([128, BLOCK], i16,
                                                   name="pdfd", tag="pdfd")
                            nc.vector.tensor_scalar(
                                pdf_t[:], dot_t[:],
                                bias_dve_sb[:, 0:1], 0.0,
                                mybir.AluOpType.add, mybir.AluOpType.max,
                            )
                            is_i16 = True
                        base = ci * (CHUNK // TILE_N)
                        pending.append(
                            (pdf_t, is_i16, gr, [base + t for t in tiles_c]))
                        if len(pending) > RED_LAG:
                            emit_reds(*pending.pop(0))
            while pending:
                emit_reds(*pending.pop(0))

    nc.compile()
    return nc


def _get_nc(repeat=1):
    global _CACHED_NC
    if repeat != 1:
        return _build_bass(repeat=repeat)
    if _CACHED_NC is None:
        _CACHED_NC = _build_bass()
    return _CACHED_NC


def _host_prep(lambdas, kappas, thetas, phis, wi):
    """Build per-core input maps (tiny O(64) parameter math + bf16 hi/lo
    split and layout of wi)."""
    lambdas = np.asarray(lambdas, np.float32)
    kappas = np.asarray(kappas, np.float32)
    thetas = np.asarray(thetas, np.float32)
    phis = np.asarray(phis, np.float32)
    wi = np.ascontiguousarray(np.asarray(wi, np.float32))

    # spherical -> cartesian mean directions, scaled by M*kappa so the
    # PSUM dot lands in Schraudolph t-units (ACT undoes with scale=ln2/128)
    st = np.sin(thetas)
    mu = np.stack([st * np.cos(phis), st * np.sin(phis), np.cos(thetas)],
                  axis=-1).astype(np.float32)          # [64, 3]
    A = (mu * (kappas * np.float32(M_SCH))[:, None]).astype(np.float32)
    A1 = A.astype(BF16)
    A2 = (A - A1.astype(np.float32)).astype(BF16)

    # vMF normalization (mirrors reference._vmf_norm, fp32)
    k = np.maximum(kappas, np.float32(1e-8))
    with np.errstate(divide="ignore", over="ignore", invalid="ignore"):
        norm_k = np.where(
            kappas < np.float32(1e-5),
            np.float32(1.0 / (4.0 * math.pi)),
            k * np.float32(1.0 / (2.0 * math.pi))
            / (np.float32(1.0) - np.exp(-2.0 * k).astype(np.float32)),
        ).astype(np.float32)
    bias64 = (np.log(lambdas * norm_k) - kappas).astype(np.float32)   # [64]
    bias128 = np.concatenate([bias64, bias64]).astype(np.float32)
    bias_act = bias128.reshape(128, 1)
    bias_dve = (bias128.astype(np.float64) * M_SCH + 16256.0
                + DELTA).astype(np.float32).reshape(128, 1)

    # lhs for the dot matmul: block-diagonal bf16 hi/lo split of A
    # pairing rows: (A1,B1) (A1,B2) (A2,B1) over the 3 dims each;
    # replicated on the four 32-row PE strips for row-tiled matmuls
    A9 = np.concatenate([A1.T, A1.T, A2.T], axis=0)    # [9, 64] bf16
    lhs18 = np.zeros((18, 128), BF16)
    lhs18[0:9, 0:64] = A9
    lhs18[9:18, 64:128] = A9
    lhs_dot = np.zeros((128, 128), BF16)
    for q in range(4):
        lhs_dot[32 * q:32 * q + 18, :] = lhs18

    # lhs for the reduction matmul: sliding-window one-hot block
    big_red = np.zeros((128, 64), BF16)
    big_red[0:64, 32] = BF16(1.0)
    big_red[64:128, 33] = BF16(1.0)

    # wi bf16 hi/lo split, paired to match lhs rows
    B1 = wi.astype(BF16)                               # [S, 3]
    B2 = (wi - B1.astype(np.float32)).astype(BF16)
    B9 = np.concatenate([B1.T, B2.T, B1.T], axis=0)    # [9, S] bf16

    in_maps = []
    for c in range(N_CORES):
        c0 = c * S_LOCAL
        wi18 = np.empty((18, S_HALF), BF16)
        wi18[0:9] = B9[:, c0:c0 + S_HALF]
        wi18[9:18] = B9[:, c0 + S_HALF:c0 + S_LOCAL]
        # scatter 512-col sample tiles over the four PE row strips
        arr = wi18.reshape(18, S_HALF // TILE_N, TILE_N)
        wi4 = np.zeros((128, S_HALF // 4), BF16)
        for q in range(4):
            wi4[32 * q:32 * q + 18] = arr[:, q::4, :].reshape(18, S_HALF // 4)
        in_maps.append({
            "wi4": wi4,
            "lhs_dot": lhs_dot,
            "big_red": big_red,
            "bias_act": bias_act,
            "bias_dve": bias_dve,
        })
    return in_maps


def _assemble(results):
    out = np.empty(N_DIRS, np.float32)
    for c in range(N_CORES):
        r = np.asarray(results[c]["out"], np.float32)   # [N_GROUPS, 128, 512]
        # PSUM partition p = 32*b + 2*i + h for sample tile t = 4*i + b,
        # half-stream h
        r = r.reshape(N_GROUPS, 4, 16, 2, TILE_N)
        c0 = c * S_LOCAL
        out[c0:c0 + S_HALF] = \
            r[:, :, :, 0, :].transpose(0, 2, 1, 3).reshape(S_HALF)
        out[c0 + S_HALF:c0 + S_LOCAL] = \
            r[:, :, :, 1, :].transpose(0, 2, 1, 3).reshape(S_HALF)
    return out


def kernel(**inputs):
    from concourse.bass_utils import run_bass_kernel_spmd

    in_maps = _host_prep(**inputs)
    nc = _get_nc()
    try:
        res = run_bass_kernel_spmd(nc, in_maps, core_ids=list(range(N_CORES)))
    except Exception:
        # one retry for transient device/terminal hiccups
        res = run_bass_kernel_spmd(nc, in_maps, core_ids=list(range(N_CORES)))
    return _assemble(res.results)


def kernel_traced(**inputs):
    """Like kernel() but with NTFF tracing; returns (out, BassKernelResults)."""
    from concourse.bass_utils import run_bass_kernel_spmd

    in_maps = _host_prep(**inputs)
    nc = _get_nc()
    res = run_bass_kernel_spmd(nc, in_maps, core_ids=list(range(N_CORES)),
                               trace=True)
    return _assemble(res.results), res
